# revision 9
# baseline (speedup 1.0000x reference)
"""Decode-style single-query attention (B=32, N=8192, D=256, H=8) on 8 TRN2 cores.

v3: single-SWDGE-queue load schedule + per-slab tiles + col-tiled PE.

- ALL loads (x slabs, constants, q rows) ride the one SWDGE (gpsimd) ring in
  exact consumption order: slab(0,0) and the prologue-critical constants
  first, then remaining consts interleaved with early slabs.  Mixing rings
  starves the small HWDGE transfers behind the 30MB slab stream (measured:
  weight casts delayed to 21-39us in v2).
- xb is one tile PER SLAB (pool bufs=18): dependency granularity is exactly
  one DMA, so slab-0 transposes start the moment slab 0 lands (v2 waited for
  the whole batch: first matmul at 49us).
- Weights are pre-cast to bf16 on the host (halves constant bytes, removes
  staging+DVE casts).
- Col-tiled (tile_position) scores and pooling; scores PSUM [128,256] per
  slab -> one wide exp + two pT transposes; zeros-matmul pre-clear for the
  shared-bank accumulators.
- Software pipeline lags: XT(g) | SC(g-1) | PT(g-2) | PL(g-3).
Measured sustained per-core DMA is ~280 GB/s -> ~120us floor for the 33.5MB
x read; everything else hides under it.
"""

import os
import sys

sys.path.insert(0, "/opt/trn_rl_repo")

from contextlib import ExitStack

import ml_dtypes
import numpy as np

import concourse.bass as bass
import concourse.tile as tile
from concourse import bacc, mybir
from concourse.bass_utils import run_bass_kernel_spmd

F32 = mybir.dt.float32
BF16 = mybir.dt.bfloat16
ts = bass.ts

B, D, H = 32, 256, 8
HPP = 32  # kq padded to 32 cols (one col-tile group)
N = 8192
DH = D // H
NCORES = 8
BL = B // NCORES
SCALE = 1.0 / float(np.sqrt(DH))

SLAB = 1024
NSUB = SLAB // 128  # 8
NSLAB = N // SLAB  # 8
G = BL * NSLAB  # 32

EXP = mybir.ActivationFunctionType.Exp

_cache = {}


def build_graph():
    nc = bacc.Bacc("TRN2", target_bir_lowering=False, debug=False, num_devices=NCORES)

    # constant blobs (bundled to minimize head-of-stream DMA count):
    # cbc (bf16, prologue+XT critical): wq 0:512 | wkT 512:1024 | id16
    #   1024:1152 | mqc 1152:1216 | zeros 1216:1344
    # cbr (bf16, epilogue): wv 0:512 | wo 512:1024 | ones 1024:1025 |
    #   msum 1025:1033
    # cf32: bqc 0:2 | bvc 2:4 | id32 rows0:32 4:36
    # mhbo (f32, 8 partitions): maskh [:,0:256] | bo [0:1,256:512]
    x_ext = nc.declare_dram_parameter("x", [BL, N, D], F32, isOutput=False)
    idz_ext = nc.declare_dram_parameter("idz", [128, 256], BF16, isOutput=False)
    cbc_ext = nc.declare_dram_parameter("cbc", [128, 1344], BF16, isOutput=False)
    cbr_ext = nc.declare_dram_parameter("cbr", [128, 1033], BF16, isOutput=False)
    cf_ext = nc.declare_dram_parameter("cf32", [128, 36], F32, isOutput=False)
    mhbo_ext = nc.declare_dram_parameter("mhbo", [H, 2 * D], F32, isOutput=False)
    out_ext = nc.declare_dram_parameter("out", [BL, D], F32, isOutput=True)

    with tile.TileContext(nc) as tc, ExitStack() as ctx:
        const = ctx.enter_context(tc.tile_pool(name="const", bufs=1))
        xbp = ctx.enter_context(tc.tile_pool(name="xb", bufs=28))
        xtp = ctx.enter_context(tc.tile_pool(name="xt", bufs=5))
        pstp = ctx.enter_context(tc.tile_pool(name="pst", bufs=6))
        ptrp = ctx.enter_context(tc.tile_pool(name="ptr", bufs=5))
        bpool = ctx.enter_context(tc.tile_pool(name="bp", bufs=1))
        ep = ctx.enter_context(tc.tile_pool(name="ep", bufs=2))
        # PSUM: xt 2 + sp 2 + pt 1 + acc 1 + eps 2 = 8 banks
        xtps = ctx.enter_context(tc.tile_pool(name="xtps", bufs=2, space="PSUM"))
        sps = ctx.enter_context(tc.tile_pool(name="sps", bufs=2, space="PSUM"))
        ptps = ctx.enter_context(tc.tile_pool(name="ptps", bufs=1, space="PSUM"))
        accp = ctx.enter_context(tc.tile_pool(name="accp", bufs=1, space="PSUM"))
        epsum = ctx.enter_context(tc.tile_pool(name="epsum", bufs=2, space="PSUM"))

        dma = nc.gpsimd  # the single ordered load queue

        # ---- tiles (constant blobs, sliced into views) ----
        idz = const.tile([128, 256], BF16)
        cbc = const.tile([128, 1344], BF16)
        cbr = const.tile([128, 1033], BF16)
        cf = const.tile([128, 36], F32)
        mhbo = const.tile([H, 2 * D], F32)

        id16_sb = idz[:, 0:128]
        zeros_sb = idz[:, 128:256]
        wq16 = cbc[:, 0:512].rearrange("p (c e) -> p c e", c=2)
        wkT16 = cbc[:, 512:1024].rearrange("p (c e) -> p c e", c=2)
        mqc_sb = cbc[:, 1152:1216].rearrange("p (c h) -> p c h", c=2)
        wv16 = cbr[:, 0:512].rearrange("p (c e) -> p c e", c=2)
        wo16 = cbr[:, 512:1024].rearrange("p (c e) -> p c e", c=2)
        ones16_sb = cbr[:, 1024:1025]
        msum_sb = cbr[:, 1025:1033]
        bqc_sb = cf[:, 0:2]
        bvc_sb = cf[:, 2:4]
        id32_sb = cf[0:HPP, 4:36]
        mh_sb = mhbo[:, 0:D]
        bo_sb = mhbo[0:1, D : 2 * D]

        st = [dict() for _ in range(BL)]
        for b in range(BL):
            st[b]["qn"] = ep.tile([1, D], F32, tag="qn", name=f"qn{b}", bufs=4)

        # 31 full slabs + the final slab split into two 512-row halves (its own
        # row mapping r = p*4+j) so the post-stream pipeline drain is shorter
        xbt = []  # per-slab tiles
        for g in range(G - 1):
            xbt.append(xbp.tile([128, NSUB, D], BF16, tag="xb", name=f"xb{g}"))
        xbh = [
            xbp.tile([128, NSUB // 2, D], BF16, tag="xbh", name=f"xbh{h}", bufs=2)
            for h in range(2)
        ]

        def load_slab(g):
            b, s = divmod(g, NSLAB)
            dma.dma_start(
                xbt[g][:],
                x_ext.ap()[b, s * SLAB : (s + 1) * SLAB, :].rearrange(
                    "(p j) d -> p j d", p=128
                ),
            )

        def load_half(h):
            base = (NSLAB - 1) * SLAB + h * (SLAB // 2)
            dma.dma_start(
                xbh[h][:],
                x_ext.ap()[BL - 1, base : base + SLAB // 2, :].rearrange(
                    "(p j) d -> p j d", p=128
                ),
            )

        # ---- the load schedule (single queue, consumption order) ----
        dma.dma_start(idz[:], idz_ext.ap())
        load_slab(0)
        dma.dma_start(cbc[:], cbc_ext.ap())
        dma.dma_start(cf[:], cf_ext.ap())
        dma.dma_start(mhbo[:], mhbo_ext.ap())
        for b in range(BL):
            dma.dma_start(st[b]["qn"][:], x_ext.ap()[b, 0:1, :])
        load_slab(1)
        dma.dma_start(cbr[:], cbr_ext.ap())
        for g in range(2, G - 1):
            load_slab(g)
        load_half(0)
        load_half(1)

        def prologue(b):
            # derive the q column [128, 2] from the q row via two PE transposes
            qt_ps = epsum.tile([128, 2], F32, tag="eps", name=f"qt_ps{b}")
            for c in range(2):
                nc.tensor.transpose(
                    qt_ps[:, c : c + 1], st[b]["qn"][:, ts(c, 128)], id32_sb[:1, :1]
                )
            qT16 = ep.tile([128, 2], BF16, tag="qT16", name=f"qT16_{b}")
            nc.vector.tensor_copy(qT16[:], qt_ps[:])
            st[b]["qbo"] = bpool.tile([1, D], F32, tag=f"qbo{b}", name=f"qbo{b}")
            nc.vector.tensor_add(st[b]["qbo"][:], st[b]["qn"][:], bo_sb[:])

            qf_ps = epsum.tile([128, 2], F32, tag="eps", name=f"qf_ps{b}")
            for mc in range(2):
                for kc in range(2):
                    nc.tensor.matmul(
                        qf_ps[:, mc : mc + 1],
                        wq16[:, kc, ts(mc, 128)],
                        qT16[:, kc : kc + 1],
                        start=(kc == 0),
                        stop=(kc == 1),
                    )
            qfb = ep.tile([128, 2], F32, tag="qfb", name=f"qfb{b}")
            nc.vector.tensor_add(qfb[:], qf_ps[:], bqc_sb[:])

            sq16 = ep.tile([128, 2, HPP], BF16, tag="sq16", name=f"sq16_{b}")
            for c in range(2):
                nc.vector.tensor_scalar_mul(
                    sq16[:, c, :], mqc_sb[:, c, :], qfb[:, c : c + 1]
                )

            kqT_ps = epsum.tile([HPP, D], F32, tag="eps", name=f"kqT_ps{b}")
            for c in range(2):
                nc.tensor.matmul(
                    kqT_ps[:], sq16[:, c, :], wkT16[:, c, :], start=(c == 0), stop=(c == 1)
                )
            kqT_sb = ep.tile([HPP, D], F32, tag="kqT", name=f"kqT{b}")
            nc.vector.tensor_copy(kqT_sb[:], kqT_ps[:])

            kq_ps = epsum.tile([128, 2, HPP], F32, tag="eps", name=f"kq_ps{b}")
            for c in range(2):
                nc.tensor.transpose(kq_ps[:, c, :], kqT_sb[:, ts(c, 128)], id32_sb[:])
            st[b]["kq16"] = bpool.tile(
                [128, 2, HPP], BF16, tag=f"kq16_{b}", name=f"kq16_{b}"
            )
            for c in range(2):
                nc.vector.tensor_copy(st[b]["kq16"][:, c, :], kq_ps[:, c, :])

            ncols = NSLAB + 1 if b == BL - 1 else NSLAB
            st[b]["lparts"] = bpool.tile([128, ncols], F32, tag=f"lp{b}", name=f"lp{b}")

        xts = {}
        pstrs = {}
        ptrs = {}

        def stage_xt(g):
            xt = xtp.tile([128, 2, NSUB, 128], BF16, tag="xt", name=f"xt{g}")
            for c in range(2):
                tp = xtps.tile([128, NSUB, 128], BF16, tag="xt", name=f"xtps{g}_{c}")
                for j in range(NSUB):
                    nc.tensor.transpose(
                        tp[:, j, :], xbt[g][:, j, ts(c, 128)], id16_sb[:]
                    )
                nc.vector.tensor_copy(xt[:, c, :, :], tp[:])
            xts[g] = xt

        def stage_scores(g):
            b, s = divmod(g, NSLAB)
            kq16 = st[b]["kq16"]
            sp = sps.tile([128, 2, 128], F32, tag="sp", name=f"sp{g}")
            spf = sp[:].rearrange("p u n -> p (u n)")
            nc.tensor.matmul(
                spf,
                zeros_sb[:],
                xts[g][:, 0, 0:2, :].rearrange("p j n -> p (j n)"),
                start=True,
                stop=True,
            )
            xtv = xts[g][:].rearrange("p c (u a) n -> p c u a n", u=2)
            for a in range(4):
                for c in range(2):
                    nc.tensor.matmul(
                        sp[32 * a : 32 * a + 32, :, :],
                        kq16[:, c, :],
                        xtv[:, c, :, a, :],
                        start=False,
                        stop=(c == 1),
                        tile_position=(0, 32 * a),
                        skip_group_check=True,
                    )
            pstr = pstp.tile([128, 2, 128], BF16, tag="ps", name=f"pstr{g}")
            nc.scalar.activation(
                pstr[:].rearrange("p u n -> p (u n)"),
                spf,
                EXP,
                scale=SCALE,
                accum_out=st[b]["lparts"][:, s : s + 1],
            )
            pstrs[g] = pstr

        def stage_pt(g):
            pt_ps = ptps.tile([128, 2, 128], BF16, tag="pt", name=f"ptps{g}")
            pstr = pstrs.pop(g)
            for u in range(2):
                nc.tensor.transpose(pt_ps[:, u, :], pstr[:, u, :], id16_sb[:])
            ptr = ptrp.tile([128, 2, 128], BF16, tag="ptr", name=f"ptr{g}")
            nc.scalar.activation(ptr[:], pt_ps[:], mybir.ActivationFunctionType.Copy)
            ptrs[g] = ptr

        def stage_pool(g):
            b, s = divmod(g, NSLAB)
            if s == 0:
                st[b]["acc"] = accp.tile([128, D], F32, tag="acc", name=f"acc{b}")
                nc.tensor.matmul(
                    st[b]["acc"][:], zeros_sb[:], wv16[:, 0, :], start=True, stop=True
                )
            acc = st[b]["acc"]
            ptr = ptrs.pop(g)
            for u in range(2):
                for a in range(4):
                    j = u * 4 + a
                    nc.tensor.matmul(
                        acc[32 * a : 32 * a + 8, :],
                        ptr[:, u, 32 * a : 32 * a + 8],
                        xbt[g][:, j, :],
                        start=False,
                        stop=(s == NSLAB - 1 and u == 1),
                        tile_position=(0, 32 * a),
                        skip_group_check=True,
                    )

        def epilogue(b):
            lsum = ep.tile([128, 1], F32, tag="lsum", name=f"lsum{b}")
            nc.vector.tensor_reduce(
                lsum[:],
                st[b]["lparts"][:],
                axis=mybir.AxisListType.X,
                op=mybir.AluOpType.add,
            )
            acs = ep.tile([128, D + 1], BF16, tag="acs", name=f"acs{b}")
            nc.vector.tensor_copy(acs[:, 0:D], st[b]["acc"][:])
            nc.vector.tensor_copy(acs[:, D : D + 1], lsum[:])

            y_ps = epsum.tile([H, D + 1], F32, tag="eps", name=f"y_ps{b}")
            nc.tensor.matmul(y_ps[:], msum_sb[:], acs[:], start=True, stop=True)

            zinv = ep.tile([H, 1], F32, tag="zinv", name=f"zinv{b}")
            nc.vector.reciprocal(zinv[:], y_ps[:, D : D + 1])
            pooled16 = ep.tile([H, D], BF16, tag="pooled", name=f"pooled{b}")
            nc.vector.tensor_scalar_mul(pooled16[:], y_ps[:, 0:D], zinv[:, 0:1])

            pt_ps = epsum.tile([128, 2, H], BF16, tag="eps", name=f"ept_ps{b}")
            for c in range(2):
                nc.tensor.transpose(
                    pt_ps[:, c, :], pooled16[:, ts(c, 128)], id16_sb[:H, :H]
                )
            pt16 = ep.tile([128, 2, H], BF16, tag="pt16", name=f"pt16_{b}")
            for c in range(2):
                nc.vector.tensor_copy(pt16[:, c, :], pt_ps[:, c, :])

            y2_ps = epsum.tile([H, D], F32, tag="eps", name=f"y2_ps{b}")
            for c in range(2):
                nc.tensor.matmul(
                    y2_ps[:], pt16[:, c, :], wv16[:, c, :], start=(c == 0), stop=(c == 1)
                )
            ym16 = ep.tile([H, D], BF16, tag="ym", name=f"ym{b}")
            nc.vector.tensor_mul(ym16[:], y2_ps[:], mh_sb[:])

            # attn^T directly: contract ym16 over heads (bv@Wo+bo folded into
            # the host-side bias, so no bias add needed here)
            at_ps = epsum.tile([128, 2], F32, tag="eps", name=f"at_ps{b}")
            for c in range(2):
                nc.tensor.matmul(
                    at_ps[:, c : c + 1],
                    ym16[:, ts(c, 128)],
                    ones16_sb[0:H, 0:1],
                    start=True,
                    stop=True,
                )
            at16 = ep.tile([128, 2], BF16, tag="at16", name=f"at16_{b}")
            nc.vector.tensor_copy(at16[:], at_ps[:])

            res_ps = epsum.tile([1, D], F32, tag="eps", name=f"res_ps{b}")
            for c in range(2):
                nc.tensor.matmul(
                    res_ps[:],
                    at16[:, c : c + 1],
                    wo16[:, c, :],
                    start=(c == 0),
                    stop=(c == 1),
                )
            out_sb = ep.tile([1, D], F32, tag="out", name=f"out{b}")
            nc.vector.tensor_add(out_sb[:], res_ps[:], st[b]["qbo"][:])
            nc.sync.dma_start(out_ext.ap()[b : b + 1, :], out_sb[:])

        # ---- half-slab stage variants (final 1024 rows) ----
        def stage_xt_h(h):
            xt = xtp.tile([128, 2, NSUB, 128], BF16, tag="xt", name=f"xth{h}")
            for c in range(2):
                tp = xtps.tile([128, NSUB, 128], BF16, tag="xt", name=f"xtpsh{h}_{c}")
                for j in range(NSUB // 2):
                    nc.tensor.transpose(
                        tp[:, j, :], xbh[h][:, j, ts(c, 128)], id16_sb[:]
                    )
                nc.vector.tensor_copy(
                    xt[:, c, 0 : NSUB // 2, :], tp[:, 0 : NSUB // 2, :]
                )
            xts[("h", h)] = xt

        def stage_scores_h(h):
            kq16 = st[BL - 1]["kq16"]
            sp = sps.tile([128, 2, 128], F32, tag="sp", name=f"sph{h}")
            xt = xts.pop(("h", h))
            nc.tensor.matmul(
                sp[:, 0, :], zeros_sb[:], xt[:, 0, 0, :], start=True, stop=True
            )
            for a in range(4):
                for c in range(2):
                    nc.tensor.matmul(
                        sp[32 * a : 32 * a + 32, 0, :],
                        kq16[:, c, :],
                        xt[:, c, a, :],
                        start=False,
                        stop=(c == 1),
                        tile_position=(0, 32 * a),
                        skip_group_check=True,
                    )
            pstr = pstp.tile([128, 2, 128], BF16, tag="ps", name=f"pstrh{h}")
            nc.scalar.activation(
                pstr[:, 0, :],
                sp[:, 0, :],
                EXP,
                scale=SCALE,
                accum_out=st[BL - 1]["lparts"][:, NSLAB - 1 + h : NSLAB + h],
            )
            pstrs[("h", h)] = pstr

        def stage_pt_h(h):
            pt_ps = ptps.tile([128, 2, 128], BF16, tag="pt", name=f"ptpsh{h}")
            nc.tensor.transpose(pt_ps[:, 0, :], pstrs.pop(("h", h))[:, 0, :], id16_sb[:])
            ptr = ptrp.tile([128, 2, 128], BF16, tag="ptr", name=f"ptrh{h}")
            nc.scalar.activation(ptr[:, 0, :], pt_ps[:, 0, :], mybir.ActivationFunctionType.Copy)
            ptrs[("h", h)] = ptr

        def stage_pool_h(h):
            acc = st[BL - 1]["acc"]
            ptr = ptrs.pop(("h", h))
            for a in range(4):
                nc.tensor.matmul(
                    acc[32 * a : 32 * a + 8, :],
                    ptr[:, 0, 32 * a : 32 * a + 8],
                    xbh[h][:, a, :],
                    start=False,
                    stop=(h == 1),
                    tile_position=(0, 32 * a),
                    skip_group_check=True,
                )

        # ---- software-pipelined main loop over units ----
        # 31 full slabs + 2 half slabs; lags: scores g-1, pt g-3, pool g-4
        units = [("f", g) for g in range(G - 1)] + [("h", 0), ("h", 1)]

        def do(stage, unit):
            kind, idx = unit
            fns = {
                ("xt", "f"): stage_xt,
                ("xt", "h"): stage_xt_h,
                ("sc", "f"): stage_scores,
                ("sc", "h"): stage_scores_h,
                ("pt", "f"): stage_pt,
                ("pt", "h"): stage_pt_h,
                ("pl", "f"): stage_pool,
                ("pl", "h"): stage_pool_h,
            }
            fns[(stage, kind)](idx)

        NU = len(units)
        for i in range(NU + 4):
            if i < NU:
                do("xt", units[i])
                if units[i][0] == "f" and units[i][1] % NSLAB == 0:
                    prologue(units[i][1] // NSLAB)
            if 1 <= i <= NU:
                do("sc", units[i - 1])
            if 3 <= i <= NU + 2:
                do("pt", units[i - 3])
            if 4 <= i <= NU + 3:
                u = units[i - 4]
                do("pl", u)
                if u[0] == "f" and u[1] % NSLAB == NSLAB - 1:
                    epilogue(u[1] // NSLAB)
                elif u == ("h", 1):
                    epilogue(BL - 1)

    nc.compile()
    return nc


def _w16(w):
    # [D, D] f32 -> [128, 2*D] bf16, contraction dim chunked onto partitions
    return (
        w.reshape(2, 128, D)
        .transpose(1, 0, 2)
        .reshape(128, 2 * D)
        .astype(ml_dtypes.bfloat16)
    )


def _host_consts(Wq, Wk, Wv, Wo, bq, bv, bo):
    e = np.arange(D)
    mq = (e[:, None] // DH == np.arange(HPP)[None, :]).astype(np.float32)  # [D, HPP]
    mqc = mq.reshape(2, 128, HPP).transpose(1, 0, 2).reshape(128, 2 * HPP)
    msum = (np.arange(128)[:, None] % 32 == np.arange(H)[None, :]) & (
        np.arange(128)[:, None] % 32 < H
    )
    cbc = np.concatenate(
        [
            _w16(Wq),
            _w16(np.ascontiguousarray(Wk.T)),
            np.eye(128, dtype=ml_dtypes.bfloat16),
            mqc.astype(ml_dtypes.bfloat16),
            np.zeros((128, 128), ml_dtypes.bfloat16),
        ],
        axis=1,
    )
    cbr = np.concatenate(
        [
            _w16(Wv),
            _w16(Wo),
            np.ones((128, 1), ml_dtypes.bfloat16),
            msum.astype(ml_dtypes.bfloat16),
        ],
        axis=1,
    )
    cf = np.zeros((128, 36), np.float32)
    cf[:, 0:2] = bq.reshape(2, 128).T
    cf[:, 2:4] = bv.reshape(2, 128).T
    cf[0:HPP, 4:36] = np.eye(HPP, dtype=np.float32)
    mhbo = np.zeros((H, 2 * D), np.float32)
    mhbo[:, 0:D] = (np.arange(H)[:, None] == e[None, :] // DH).astype(np.float32)
    mhbo[0, D : 2 * D] = bo + bv @ Wo  # residual bias: attn@Wo+bo = pooledV@Wo + (bv@Wo+bo)
    idz = np.concatenate(
        [np.eye(128, dtype=ml_dtypes.bfloat16), np.zeros((128, 128), ml_dtypes.bfloat16)],
        axis=1,
    )
    return {
        "idz": np.ascontiguousarray(idz),
        "cbc": np.ascontiguousarray(cbc),
        "cbr": np.ascontiguousarray(cbr),
        "cf32": cf,
        "mhbo": mhbo,
    }


def kernel(**inputs):
    x = np.ascontiguousarray(np.asarray(inputs["x"], dtype=np.float32))
    Wq = np.asarray(inputs["Wq"], dtype=np.float32)
    bq = np.asarray(inputs["bq"], dtype=np.float32)
    Wk = np.asarray(inputs["Wk"], dtype=np.float32)
    Wv = np.asarray(inputs["Wv"], dtype=np.float32)
    Wo = np.asarray(inputs["Wo"], dtype=np.float32)
    bv = np.asarray(inputs["bv"], dtype=np.float32)
    bo = np.asarray(inputs["bo"], dtype=np.float32)
    # bk is unused: softmax is shift-invariant and Q.bk is constant over keys.

    if "nc" not in _cache:
        _cache["nc"] = build_graph()
    nc = _cache["nc"]

    shared = _host_consts(Wq, Wk, Wv, Wo, bq, bv, bo)
    in_maps = []
    for c in range(NCORES):
        m = dict(shared)
        m["x"] = np.ascontiguousarray(x[c * BL : (c + 1) * BL])
        in_maps.append(m)

    trace = bool(int(os.environ.get("K_TRACE", "0")))
    res = run_bass_kernel_spmd(
        nc,
        in_maps,
        core_ids=list(range(NCORES)),
        trace=trace,
        tmpdir=os.environ.get("K_TRACE_DIR") or None,
    )
    _cache["last_results"] = res
    out = np.concatenate([res.results[i]["out"] for i in range(NCORES)], axis=0)
    return out.reshape(B, 1, D).astype(np.float32)


# revision 11
# speedup vs baseline: 1.0391x; 1.0391x over previous
"""Decode-style single-query attention (B=32, N=8192, D=256, H=8) on 8 TRN2 cores.

v3: single-SWDGE-queue load schedule + per-slab tiles + col-tiled PE.

- ALL loads (x slabs, constants, q rows) ride the one SWDGE (gpsimd) ring in
  exact consumption order: slab(0,0) and the prologue-critical constants
  first, then remaining consts interleaved with early slabs.  Mixing rings
  starves the small HWDGE transfers behind the 30MB slab stream (measured:
  weight casts delayed to 21-39us in v2).
- xb is one tile PER SLAB (pool bufs=18): dependency granularity is exactly
  one DMA, so slab-0 transposes start the moment slab 0 lands (v2 waited for
  the whole batch: first matmul at 49us).
- Weights are pre-cast to bf16 on the host (halves constant bytes, removes
  staging+DVE casts).
- Col-tiled (tile_position) scores and pooling; scores PSUM [128,256] per
  slab -> one wide exp + two pT transposes; zeros-matmul pre-clear for the
  shared-bank accumulators.
- Software pipeline lags: XT(g) | SC(g-1) | PT(g-2) | PL(g-3).
Measured sustained per-core DMA is ~280 GB/s -> ~120us floor for the 33.5MB
x read; everything else hides under it.
"""

import os
import sys

sys.path.insert(0, "/opt/trn_rl_repo")

from contextlib import ExitStack

import ml_dtypes
import numpy as np

import concourse.bass as bass
import concourse.tile as tile
from concourse import bacc, mybir
from concourse.bass_utils import run_bass_kernel_spmd

F32 = mybir.dt.float32
BF16 = mybir.dt.bfloat16
ts = bass.ts

B, D, H = 32, 256, 8
HPP = 32  # kq padded to 32 cols (one col-tile group)
N = 8192
DH = D // H
NCORES = 8
BL = B // NCORES
SCALE = 1.0 / float(np.sqrt(DH))

SLAB = 1024
NSUB = SLAB // 128  # 8
NSLAB = N // SLAB  # 8
G = BL * NSLAB  # 32

EXP = mybir.ActivationFunctionType.Exp

_cache = {}


def build_graph():
    nc = bacc.Bacc("TRN2", target_bir_lowering=False, debug=False, num_devices=NCORES)

    # constant blobs (bundled to minimize head-of-stream DMA count):
    # cbc (bf16, prologue+XT critical): wq 0:512 | wkT 512:1024 | id16
    #   1024:1152 | mqc 1152:1216 | zeros 1216:1344
    # cbr (bf16, epilogue): wv 0:512 | wo 512:1024 | ones 1024:1025 |
    #   msum 1025:1033
    # cf32: bqc 0:2 | bvc 2:4 | id32 rows0:32 4:36
    # mhbo (f32, 8 partitions): maskh [:,0:256] | bo [0:1,256:512]
    x_ext = nc.declare_dram_parameter("x", [BL, N, D], F32, isOutput=False)
    idz_ext = nc.declare_dram_parameter("idz", [128, 256], BF16, isOutput=False)
    cbc_ext = nc.declare_dram_parameter("cbc", [128, 1344], BF16, isOutput=False)
    cbr_ext = nc.declare_dram_parameter("cbr", [128, 1033], BF16, isOutput=False)
    cf_ext = nc.declare_dram_parameter("cf32", [128, 36], F32, isOutput=False)
    mhbo_ext = nc.declare_dram_parameter("mhbo", [H, 2 * D], F32, isOutput=False)
    out_ext = nc.declare_dram_parameter("out", [BL, D], F32, isOutput=True)

    with tile.TileContext(nc) as tc, ExitStack() as ctx:
        const = ctx.enter_context(tc.tile_pool(name="const", bufs=1))
        xbp = ctx.enter_context(tc.tile_pool(name="xb", bufs=28))
        xtp = ctx.enter_context(tc.tile_pool(name="xt", bufs=5))
        pstp = ctx.enter_context(tc.tile_pool(name="pst", bufs=6))
        ptrp = ctx.enter_context(tc.tile_pool(name="ptr", bufs=5))
        bpool = ctx.enter_context(tc.tile_pool(name="bp", bufs=1))
        ep = ctx.enter_context(tc.tile_pool(name="ep", bufs=2))
        # PSUM: xt 2 + sp 2 + pt 1 + acc 1 + eps 2 = 8 banks
        xtps = ctx.enter_context(tc.tile_pool(name="xtps", bufs=2, space="PSUM"))
        sps = ctx.enter_context(tc.tile_pool(name="sps", bufs=2, space="PSUM"))
        ptps = ctx.enter_context(tc.tile_pool(name="ptps", bufs=1, space="PSUM"))
        accp = ctx.enter_context(tc.tile_pool(name="accp", bufs=1, space="PSUM"))
        epsum = ctx.enter_context(tc.tile_pool(name="epsum", bufs=2, space="PSUM"))

        dma = nc.gpsimd  # the single ordered load queue

        # ---- tiles (constant blobs, sliced into views) ----
        idz = const.tile([128, 256], BF16)
        cbc = const.tile([128, 1344], BF16)
        cbr = const.tile([128, 1033], BF16)
        cf = const.tile([128, 36], F32)
        mhbo = const.tile([H, 2 * D], F32)

        id16_sb = idz[:, 0:128]
        zeros_sb = idz[:, 128:256]
        wq16 = cbc[:, 0:512].rearrange("p (c e) -> p c e", c=2)
        wkT16 = cbc[:, 512:1024].rearrange("p (c e) -> p c e", c=2)
        mqc_sb = cbc[:, 1152:1216].rearrange("p (c h) -> p c h", c=2)
        wv16 = cbr[:, 0:512].rearrange("p (c e) -> p c e", c=2)
        wo16 = cbr[:, 512:1024].rearrange("p (c e) -> p c e", c=2)
        ones16_sb = cbr[:, 1024:1025]
        msum_sb = cbr[:, 1025:1033]
        bqc_sb = cf[:, 0:2]
        bvc_sb = cf[:, 2:4]
        id32_sb = cf[0:HPP, 4:36]
        mh_sb = mhbo[:, 0:D]
        bo_sb = mhbo[0:1, D : 2 * D]

        st = [dict() for _ in range(BL)]
        for b in range(BL):
            st[b]["qn"] = ep.tile([1, D], F32, tag="qn", name=f"qn{b}", bufs=4)

        # 31 full slabs + the final slab split into two 512-row halves (its own
        # row mapping r = p*4+j) so the post-stream pipeline drain is shorter
        xbt = []  # per-slab tiles
        for g in range(G - 1):
            xbt.append(xbp.tile([128, NSUB, D], BF16, tag="xb", name=f"xb{g}"))
        xbh = [
            xbp.tile([128, NSUB // 2, D], BF16, tag="xbh", name=f"xbh{h}", bufs=2)
            for h in range(2)
        ]

        def load_slab(g):
            b, s = divmod(g, NSLAB)
            dma.dma_start(
                xbt[g][:],
                x_ext.ap()[b, s * SLAB : (s + 1) * SLAB, :].rearrange(
                    "(p j) d -> p j d", p=128
                ),
            )

        def load_half(h):
            base = (NSLAB - 1) * SLAB + h * (SLAB // 2)
            dma.dma_start(
                xbh[h][:],
                x_ext.ap()[BL - 1, base : base + SLAB // 2, :].rearrange(
                    "(p j) d -> p j d", p=128
                ),
            )

        # ---- the load schedule (single queue, consumption order) ----
        dma.dma_start(idz[:], idz_ext.ap())
        load_slab(0)
        dma.dma_start(cbc[:], cbc_ext.ap())
        dma.dma_start(cf[:], cf_ext.ap())
        dma.dma_start(mhbo[:], mhbo_ext.ap())
        for b in range(BL):
            dma.dma_start(st[b]["qn"][:], x_ext.ap()[b, 0:1, :])
        load_slab(1)
        dma.dma_start(cbr[:], cbr_ext.ap())
        for g in range(2, G - 1):
            load_slab(g)
        load_half(0)
        load_half(1)

        def prologue(b):
            # derive the q column [128, 2] from the q row via two PE transposes
            qt_ps = epsum.tile([128, 2], F32, tag="eps", name=f"qt_ps{b}")
            for c in range(2):
                nc.tensor.transpose(
                    qt_ps[:, c : c + 1], st[b]["qn"][:, ts(c, 128)], id32_sb[:1, :1]
                )
            qT16 = ep.tile([128, 2], BF16, tag="qT16", name=f"qT16_{b}")
            nc.vector.tensor_copy(qT16[:], qt_ps[:])
            st[b]["qbo"] = bpool.tile([1, D], F32, tag=f"qbo{b}", name=f"qbo{b}")
            nc.vector.tensor_add(st[b]["qbo"][:], st[b]["qn"][:], bo_sb[:])

            qf_ps = epsum.tile([128, 2], F32, tag="eps", name=f"qf_ps{b}")
            for mc in range(2):
                for kc in range(2):
                    nc.tensor.matmul(
                        qf_ps[:, mc : mc + 1],
                        wq16[:, kc, ts(mc, 128)],
                        qT16[:, kc : kc + 1],
                        start=(kc == 0),
                        stop=(kc == 1),
                    )
            qfb = ep.tile([128, 2], F32, tag="qfb", name=f"qfb{b}")
            nc.vector.tensor_add(qfb[:], qf_ps[:], bqc_sb[:])

            sq16 = ep.tile([128, 2, HPP], BF16, tag="sq16", name=f"sq16_{b}")
            for c in range(2):
                nc.vector.tensor_scalar_mul(
                    sq16[:, c, :], mqc_sb[:, c, :], qfb[:, c : c + 1]
                )

            kqT_ps = epsum.tile([HPP, D], F32, tag="eps", name=f"kqT_ps{b}")
            for c in range(2):
                nc.tensor.matmul(
                    kqT_ps[:], sq16[:, c, :], wkT16[:, c, :], start=(c == 0), stop=(c == 1)
                )
            kqT_sb = ep.tile([HPP, D], F32, tag="kqT", name=f"kqT{b}")
            nc.vector.tensor_copy(kqT_sb[:], kqT_ps[:])

            kq_ps = epsum.tile([128, 2, HPP], F32, tag="eps", name=f"kq_ps{b}")
            for c in range(2):
                nc.tensor.transpose(kq_ps[:, c, :], kqT_sb[:, ts(c, 128)], id32_sb[:])
            st[b]["kq16"] = bpool.tile(
                [128, 2, HPP], BF16, tag=f"kq16_{b}", name=f"kq16_{b}"
            )
            for c in range(2):
                nc.vector.tensor_copy(st[b]["kq16"][:, c, :], kq_ps[:, c, :])

            ncols = NSLAB + 1 if b == BL - 1 else NSLAB
            st[b]["lparts"] = bpool.tile([128, ncols], F32, tag=f"lp{b}", name=f"lp{b}")

        xts = {}
        pstrs = {}
        ptrs = {}

        def stage_xt(g):
            xt = xtp.tile([128, 2, NSUB, 128], BF16, tag="xt", name=f"xt{g}")
            for c in range(2):
                tp = xtps.tile([128, NSUB, 128], BF16, tag="xt", name=f"xtps{g}_{c}")
                for j in range(NSUB):
                    nc.tensor.transpose(
                        tp[:, j, :], xbt[g][:, j, ts(c, 128)], id16_sb[:]
                    )
                nc.vector.tensor_copy(xt[:, c, :, :], tp[:])
            xts[g] = xt

        def stage_scores(g):
            b, s = divmod(g, NSLAB)
            kq16 = st[b]["kq16"]
            sp = sps.tile([128, 2, 128], F32, tag="sp", name=f"sp{g}")
            spf = sp[:].rearrange("p u n -> p (u n)")
            nc.tensor.matmul(
                spf,
                zeros_sb[:],
                xts[g][:, 0, 0:2, :].rearrange("p j n -> p (j n)"),
                start=True,
                stop=True,
            )
            xtv = xts[g][:].rearrange("p c (u a) n -> p c u a n", u=2)
            for a in range(4):
                for c in range(2):
                    nc.tensor.matmul(
                        sp[32 * a : 32 * a + 32, :, :],
                        kq16[:, c, :],
                        xtv[:, c, :, a, :],
                        start=False,
                        stop=(c == 1),
                        tile_position=(0, 32 * a),
                        skip_group_check=True,
                    )
            pstr = pstp.tile([128, 2, 128], BF16, tag="ps", name=f"pstr{g}")
            nc.scalar.activation(
                pstr[:].rearrange("p u n -> p (u n)"),
                spf,
                EXP,
                scale=SCALE,
                accum_out=st[b]["lparts"][:, s : s + 1],
            )
            pstrs[g] = pstr

        def stage_pt(g):
            pt_ps = ptps.tile([128, 2, 128], BF16, tag="pt", name=f"ptps{g}")
            pstr = pstrs.pop(g)
            for u in range(2):
                nc.tensor.transpose(pt_ps[:, u, :], pstr[:, u, :], id16_sb[:])
            ptr = ptrp.tile([128, 2, 128], BF16, tag="ptr", name=f"ptr{g}")
            nc.vector.tensor_copy(ptr[:], pt_ps[:])
            ptrs[g] = ptr

        def stage_pool(g):
            b, s = divmod(g, NSLAB)
            if s == 0:
                st[b]["acc"] = accp.tile([128, D], F32, tag="acc", name=f"acc{b}")
                nc.tensor.matmul(
                    st[b]["acc"][:], zeros_sb[:], wv16[:, 0, :], start=True, stop=True
                )
            acc = st[b]["acc"]
            ptr = ptrs.pop(g)
            for u in range(2):
                for a in range(4):
                    j = u * 4 + a
                    nc.tensor.matmul(
                        acc[32 * a : 32 * a + 8, :],
                        ptr[:, u, 32 * a : 32 * a + 8],
                        xbt[g][:, j, :],
                        start=False,
                        stop=(s == NSLAB - 1 and u == 1),
                        tile_position=(0, 32 * a),
                        skip_group_check=True,
                    )

        def epilogue(b):
            lsum = ep.tile([128, 1], F32, tag="lsum", name=f"lsum{b}")
            nc.vector.tensor_reduce(
                lsum[:],
                st[b]["lparts"][:],
                axis=mybir.AxisListType.X,
                op=mybir.AluOpType.add,
            )
            acs = ep.tile([128, D + 1], BF16, tag="acs", name=f"acs{b}")
            nc.vector.tensor_copy(acs[:, 0:D], st[b]["acc"][:])
            nc.vector.tensor_copy(acs[:, D : D + 1], lsum[:])

            y_ps = epsum.tile([H, D + 1], F32, tag="eps", name=f"y_ps{b}")
            nc.tensor.matmul(y_ps[:], msum_sb[:], acs[:], start=True, stop=True)

            zinv = ep.tile([H, 1], F32, tag="zinv", name=f"zinv{b}")
            nc.vector.reciprocal(zinv[:], y_ps[:, D : D + 1])
            pooled16 = ep.tile([H, D], BF16, tag="pooled", name=f"pooled{b}")
            nc.vector.tensor_scalar_mul(pooled16[:], y_ps[:, 0:D], zinv[:, 0:1])

            pt_ps = epsum.tile([128, 2, H], BF16, tag="eps", name=f"ept_ps{b}")
            for c in range(2):
                nc.tensor.transpose(
                    pt_ps[:, c, :], pooled16[:, ts(c, 128)], id16_sb[:H, :H]
                )
            pt16 = ep.tile([128, 2, H], BF16, tag="pt16", name=f"pt16_{b}")
            for c in range(2):
                nc.vector.tensor_copy(pt16[:, c, :], pt_ps[:, c, :])

            y2_ps = epsum.tile([H, D], F32, tag="eps", name=f"y2_ps{b}")
            for c in range(2):
                nc.tensor.matmul(
                    y2_ps[:], pt16[:, c, :], wv16[:, c, :], start=(c == 0), stop=(c == 1)
                )
            ym16 = ep.tile([H, D], BF16, tag="ym", name=f"ym{b}")
            nc.vector.tensor_mul(ym16[:], y2_ps[:], mh_sb[:])

            # attn^T directly: contract ym16 over heads (bv@Wo+bo folded into
            # the host-side bias, so no bias add needed here)
            at_ps = epsum.tile([128, 2], F32, tag="eps", name=f"at_ps{b}")
            for c in range(2):
                nc.tensor.matmul(
                    at_ps[:, c : c + 1],
                    ym16[:, ts(c, 128)],
                    ones16_sb[0:H, 0:1],
                    start=True,
                    stop=True,
                )
            at16 = ep.tile([128, 2], BF16, tag="at16", name=f"at16_{b}")
            nc.vector.tensor_copy(at16[:], at_ps[:])

            res_ps = epsum.tile([1, D], F32, tag="eps", name=f"res_ps{b}")
            for c in range(2):
                nc.tensor.matmul(
                    res_ps[:],
                    at16[:, c : c + 1],
                    wo16[:, c, :],
                    start=(c == 0),
                    stop=(c == 1),
                )
            out_sb = ep.tile([1, D], F32, tag="out", name=f"out{b}")
            nc.vector.tensor_add(out_sb[:], res_ps[:], st[b]["qbo"][:])
            nc.sync.dma_start(out_ext.ap()[b : b + 1, :], out_sb[:])

        # ---- half-slab stage variants (final 1024 rows) ----
        def stage_xt_h(h):
            xt = xtp.tile([128, 2, NSUB, 128], BF16, tag="xt", name=f"xth{h}")
            for c in range(2):
                tp = xtps.tile([128, NSUB, 128], BF16, tag="xt", name=f"xtpsh{h}_{c}")
                for j in range(NSUB // 2):
                    nc.tensor.transpose(
                        tp[:, j, :], xbh[h][:, j, ts(c, 128)], id16_sb[:]
                    )
                nc.vector.tensor_copy(
                    xt[:, c, 0 : NSUB // 2, :], tp[:, 0 : NSUB // 2, :]
                )
            xts[("h", h)] = xt

        def stage_scores_h(h):
            kq16 = st[BL - 1]["kq16"]
            sp = sps.tile([128, 2, 128], F32, tag="sp", name=f"sph{h}")
            xt = xts.pop(("h", h))
            nc.tensor.matmul(
                sp[:, 0, :], zeros_sb[:], xt[:, 0, 0, :], start=True, stop=True
            )
            for a in range(4):
                for c in range(2):
                    nc.tensor.matmul(
                        sp[32 * a : 32 * a + 32, 0, :],
                        kq16[:, c, :],
                        xt[:, c, a, :],
                        start=False,
                        stop=(c == 1),
                        tile_position=(0, 32 * a),
                        skip_group_check=True,
                    )
            pstr = pstp.tile([128, 2, 128], BF16, tag="ps", name=f"pstrh{h}")
            nc.scalar.activation(
                pstr[:, 0, :],
                sp[:, 0, :],
                EXP,
                scale=SCALE,
                accum_out=st[BL - 1]["lparts"][:, NSLAB - 1 + h : NSLAB + h],
            )
            pstrs[("h", h)] = pstr

        def stage_pt_h(h):
            pt_ps = ptps.tile([128, 2, 128], BF16, tag="pt", name=f"ptpsh{h}")
            nc.tensor.transpose(pt_ps[:, 0, :], pstrs.pop(("h", h))[:, 0, :], id16_sb[:])
            ptr = ptrp.tile([128, 2, 128], BF16, tag="ptr", name=f"ptrh{h}")
            nc.vector.tensor_copy(ptr[:, 0, :], pt_ps[:, 0, :])
            ptrs[("h", h)] = ptr

        def stage_pool_h(h):
            acc = st[BL - 1]["acc"]
            ptr = ptrs.pop(("h", h))
            for a in range(4):
                nc.tensor.matmul(
                    acc[32 * a : 32 * a + 8, :],
                    ptr[:, 0, 32 * a : 32 * a + 8],
                    xbh[h][:, a, :],
                    start=False,
                    stop=(h == 1),
                    tile_position=(0, 32 * a),
                    skip_group_check=True,
                )

        # ---- software-pipelined main loop over units ----
        # 31 full slabs + 2 half slabs; lags: scores g-1, pt g-3, pool g-4
        units = [("f", g) for g in range(G - 1)] + [("h", 0), ("h", 1)]

        def do(stage, unit):
            kind, idx = unit
            fns = {
                ("xt", "f"): stage_xt,
                ("xt", "h"): stage_xt_h,
                ("sc", "f"): stage_scores,
                ("sc", "h"): stage_scores_h,
                ("pt", "f"): stage_pt,
                ("pt", "h"): stage_pt_h,
                ("pl", "f"): stage_pool,
                ("pl", "h"): stage_pool_h,
            }
            fns[(stage, kind)](idx)

        NU = len(units)
        for i in range(NU + 4):
            if i < NU:
                do("xt", units[i])
                if units[i][0] == "f" and units[i][1] % NSLAB == 0:
                    prologue(units[i][1] // NSLAB)
            # pt BEFORE sc: its DVE copy then lands early in the iteration,
            # so pool's LDWEIGHTS (reading ptr) never head-blocks the PE queue
            if 3 <= i <= NU + 2:
                do("pt", units[i - 3])
            if 1 <= i <= NU:
                do("sc", units[i - 1])
            if 4 <= i <= NU + 3:
                u = units[i - 4]
                do("pl", u)
                if u[0] == "f" and u[1] % NSLAB == NSLAB - 1:
                    epilogue(u[1] // NSLAB)
                elif u == ("h", 1):
                    epilogue(BL - 1)

    nc.compile()
    return nc


def _w16(w):
    # [D, D] f32 -> [128, 2*D] bf16, contraction dim chunked onto partitions
    return (
        w.reshape(2, 128, D)
        .transpose(1, 0, 2)
        .reshape(128, 2 * D)
        .astype(ml_dtypes.bfloat16)
    )


def _host_consts(Wq, Wk, Wv, Wo, bq, bv, bo):
    e = np.arange(D)
    mq = (e[:, None] // DH == np.arange(HPP)[None, :]).astype(np.float32)  # [D, HPP]
    mqc = mq.reshape(2, 128, HPP).transpose(1, 0, 2).reshape(128, 2 * HPP)
    msum = (np.arange(128)[:, None] % 32 == np.arange(H)[None, :]) & (
        np.arange(128)[:, None] % 32 < H
    )
    cbc = np.concatenate(
        [
            _w16(Wq),
            _w16(np.ascontiguousarray(Wk.T)),
            np.eye(128, dtype=ml_dtypes.bfloat16),
            mqc.astype(ml_dtypes.bfloat16),
            np.zeros((128, 128), ml_dtypes.bfloat16),
        ],
        axis=1,
    )
    cbr = np.concatenate(
        [
            _w16(Wv),
            _w16(Wo),
            np.ones((128, 1), ml_dtypes.bfloat16),
            msum.astype(ml_dtypes.bfloat16),
        ],
        axis=1,
    )
    cf = np.zeros((128, 36), np.float32)
    cf[:, 0:2] = bq.reshape(2, 128).T
    cf[:, 2:4] = bv.reshape(2, 128).T
    cf[0:HPP, 4:36] = np.eye(HPP, dtype=np.float32)
    mhbo = np.zeros((H, 2 * D), np.float32)
    mhbo[:, 0:D] = (np.arange(H)[:, None] == e[None, :] // DH).astype(np.float32)
    mhbo[0, D : 2 * D] = bo + bv @ Wo  # residual bias: attn@Wo+bo = pooledV@Wo + (bv@Wo+bo)
    idz = np.concatenate(
        [np.eye(128, dtype=ml_dtypes.bfloat16), np.zeros((128, 128), ml_dtypes.bfloat16)],
        axis=1,
    )
    return {
        "idz": np.ascontiguousarray(idz),
        "cbc": np.ascontiguousarray(cbc),
        "cbr": np.ascontiguousarray(cbr),
        "cf32": cf,
        "mhbo": mhbo,
    }


def kernel(**inputs):
    x = np.ascontiguousarray(np.asarray(inputs["x"], dtype=np.float32))
    Wq = np.asarray(inputs["Wq"], dtype=np.float32)
    bq = np.asarray(inputs["bq"], dtype=np.float32)
    Wk = np.asarray(inputs["Wk"], dtype=np.float32)
    Wv = np.asarray(inputs["Wv"], dtype=np.float32)
    Wo = np.asarray(inputs["Wo"], dtype=np.float32)
    bv = np.asarray(inputs["bv"], dtype=np.float32)
    bo = np.asarray(inputs["bo"], dtype=np.float32)
    # bk is unused: softmax is shift-invariant and Q.bk is constant over keys.

    if "nc" not in _cache:
        _cache["nc"] = build_graph()
    nc = _cache["nc"]

    shared = _host_consts(Wq, Wk, Wv, Wo, bq, bv, bo)
    in_maps = []
    for c in range(NCORES):
        m = dict(shared)
        m["x"] = np.ascontiguousarray(x[c * BL : (c + 1) * BL])
        in_maps.append(m)

    trace = bool(int(os.environ.get("K_TRACE", "0")))
    res = run_bass_kernel_spmd(
        nc,
        in_maps,
        core_ids=list(range(NCORES)),
        trace=trace,
        tmpdir=os.environ.get("K_TRACE_DIR") or None,
    )
    _cache["last_results"] = res
    out = np.concatenate([res.results[i]["out"] for i in range(NCORES)], axis=0)
    return out.reshape(B, 1, D).astype(np.float32)
